# revision 7
# baseline (speedup 1.0000x reference)
"""Trainium2 Bass kernel for nn_EncoderBlock (B=4, S=2048, D=1024, H=16).

Sharding: 8 cores = (batch b, seq-half p).  Each core computes attention for
its 1024 query tokens over the full 2048-token sequence of its batch (K/V
projections computed redundantly within the pair), then LN1 + FFN + LN2 for
its tokens.  No collectives.  Everything runs feature-major on-chip; the host
pre-transposes inputs and post-transposes the output (free for HW time).

All matmuls use float32r (full PE rate at N>=512, ~tf32 precision).
Softmax skips the max-subtraction (scores ~ N(0,1), exp is safe in fp32) and
gets denominators for free via a ones-augmented V (M=65 matmuls).
"""

import numpy as np

B, S, D = 4, 2048, 1024
H, HD = 16, 64
P = 128
DB = D // P            # 8 feature blocks of 128
TOK = S // 2           # 1024 own tokens per core
JB = S // P            # 16 key blocks
NI = TOK // 512        # 2 query i-tiles of 512
LN_EPS = 1e-5
N_CORES = 8

_cache = {}
_POOL_CMS = {}


def _pool(tc, **kw):
    cm = tc.tile_pool(**kw)
    p = cm.__enter__()
    _POOL_CMS[id(p)] = cm
    return p


def _close(p):
    _POOL_CMS.pop(id(p)).__exit__(None, None, None)


def _build():
    import concourse.tile as tile
    from concourse import bacc, mybir

    nc = bacc.Bacc("TRN2")
    f32 = mybir.dt.float32
    f32r = mybir.dt.float32r
    AF = mybir.ActivationFunctionType
    ALU = mybir.AluOpType

    ein, eout = "ExternalInput", "ExternalOutput"
    xT_d = nc.dram_tensor("xT", [D, S], f32r, kind=ein)
    xTq_d = nc.dram_tensor("xTq", [D, TOK], f32r, kind=ein)
    xTr_d = nc.dram_tensor("xTr", [D, TOK], f32, kind=ein)
    wqT_d = nc.dram_tensor("wqT", [D, D], f32r, kind=ein)
    wkT_d = nc.dram_tensor("wkT", [D, D], f32r, kind=ein)
    wvT_d = nc.dram_tensor("wvT", [D, D], f32r, kind=ein)
    l1T_d = nc.dram_tensor("l1T", [D, D], f32r, kind=ein)
    l2T_d = nc.dram_tensor("l2T", [D, D], f32r, kind=ein)
    onesr_d = nc.dram_tensor("onesr", [1, P], f32r, kind=ein)
    onesc_d = nc.dram_tensor("onesc", [P, 1], f32r, kind=ein)
    b1_d = nc.dram_tensor("b1s", [P, DB], f32, kind=ein)
    b2_d = nc.dram_tensor("b2s", [P, DB], f32, kind=ein)
    g1_d = nc.dram_tensor("g1s", [P, DB], f32, kind=ein)
    e1_d = nc.dram_tensor("e1s", [P, DB], f32, kind=ein)
    g2_d = nc.dram_tensor("g2s", [P, DB], f32, kind=ein)
    e2_d = nc.dram_tensor("e2s", [P, DB], f32, kind=ein)
    out_d = nc.dram_tensor("outT", [D, TOK], f32, kind=eout)
    ksp_d = nc.dram_tensor("ksp", [D, S], f32r)     # spilled K^T
    qsp_d = nc.dram_tensor("qsp", [D, TOK], f32r)   # spilled Q^T
    asp_d = nc.dram_tensor("asp", [D, TOK], f32)    # spilled attn^T

    def r3(dram):  # [D, t] dram -> [p, db, t] view
        return dram[:].rearrange("(db p) t -> p db t", p=P)

    with tile.TileContext(nc) as tc:
        with nc.allow_low_precision(reason="fp32r matmul inputs"):
            _body(nc, tc, f32, f32r, AF, ALU, r3,
                  xT_d, xTq_d, xTr_d, wqT_d, wkT_d, wvT_d, l1T_d, l2T_d,
                  onesr_d, onesc_d, b1_d, b2_d, g1_d, e1_d, g2_d, e2_d,
                  out_d, ksp_d, qsp_d, asp_d)
    nc.compile()
    return nc


def _body(nc, tc, f32, f32r, AF, ALU, r3,
          xT_d, xTq_d, xTr_d, wqT_d, wkT_d, wvT_d, l1T_d, l2T_d,
          onesr_d, onesc_d, b1_d, b2_d, g1_d, e1_d, g2_d, e2_d,
          out_d, ksp_d, qsp_d, asp_d):
    # ---------- persistent constants ----------
    const = _pool(tc, name="const", bufs=1)
    onesr = const.tile([1, P], f32r, tag="onesr")
    onesc = const.tile([P, 1], f32r, tag="onesc")
    nc.sync.dma_start(onesr[:], onesr_d[:])
    nc.sync.dma_start(onesc[:], onesc_d[:])
    b1s = const.tile([P, DB], f32, tag="b1s")
    b2s = const.tile([P, DB], f32, tag="b2s")
    g1s = const.tile([P, DB], f32, tag="g1s")
    e1s = const.tile([P, DB], f32, tag="e1s")
    g2s = const.tile([P, DB], f32, tag="g2s")
    e2s = const.tile([P, DB], f32, tag="e2s")
    for t, d in ((b1s, b1_d), (b2s, b2_d), (g1s, g1_d),
                 (e1s, e1_d), (g2s, g2_d), (e2s, e2_d)):
        nc.sync.dma_start(t[:], d[:])
    eps_t = const.tile([1, 1], f32, tag="eps")
    nc.vector.memset(eps_t[:], LN_EPS)

    # v_aug: [tok-in-block, key-block, head, 64 v feats + ones]
    pool_v = _pool(tc, name="vaug", bufs=1)
    v_aug = pool_v.tile([P, JB, H, HD + 1], f32r, tag="vaug")
    nc.vector.tensor_copy(
        out=v_aug[:, :, :, HD:HD + 1],
        in_=onesc[:, 0:1, None, None].to_broadcast((P, JB, H, 1)),
    )

    # ================= phase 1: V / K / Q projections =================
    pool_w = _pool(tc, name="wts", bufs=1)
    psum_p = _pool(tc, name="pj", bufs=4, space="PSUM")
    evict = _pool(tc, name="evict", bufs=3)

    pool_x = _pool(tc, name="xfull", bufs=1)
    x_sb = pool_x.tile([P, DB, S], f32r, tag="x")
    for db in range(DB):
        nc.sync.dma_start(x_sb[:, db, :], r3(xT_d)[:, db, :])

    # --- V: v[t, f] = sum_d xT[d, t-blk] * wvT[d, f] (token-major out) ---
    wv = pool_w.tile([P, DB, D], f32r, tag="w")
    for db in range(DB):
        nc.sync.dma_start(wv[:, db, :], r3(wvT_d)[:, db, :])
    for tb in range(JB):
        for fc in range(2):
            ps = psum_p.tile([P, 512], f32, tag="pj", name=f"psv{tb}_{fc}")
            for db in range(DB):
                nc.tensor.matmul(
                    ps[:], x_sb[:, db, tb * P:(tb + 1) * P],
                    wv[:, db, fc * 512:(fc + 1) * 512],
                    start=(db == 0), stop=(db == DB - 1))
            nc.vector.tensor_copy(
                out=v_aug[:, tb, fc * 8:(fc + 1) * 8, 0:HD],
                in_=ps[:].rearrange("p (h e) -> p h e", e=HD))

    # --- K: kT[f, j] = sum_d wkT[d, f-blk] * xT[d, j]; spill to DRAM ---
    wk = pool_w.tile([P, DB, D], f32r, tag="w")
    for db in range(DB):
        nc.sync.dma_start(wk[:, db, :], r3(wkT_d)[:, db, :])
    for ft in range(DB):
        for tck in range(4):
            ps = psum_p.tile([P, 512], f32, tag="pj", name=f"psk{ft}_{tck}")
            for db in range(DB):
                nc.tensor.matmul(
                    ps[:], wk[:, db, ft * P:(ft + 1) * P],
                    x_sb[:, db, tck * 512:(tck + 1) * 512],
                    start=(db == 0), stop=(db == DB - 1))
            kst = evict.tile([P, 512], f32r, tag="kst", name=f"kst{ft}_{tck}")
            nc.vector.tensor_copy(kst[:], ps[:])
            nc.sync.dma_start(
                ksp_d[ft * P:(ft + 1) * P, tck * 512:(tck + 1) * 512], kst[:])
    _close(pool_x)

    # --- Q (own tokens): spill to DRAM ---
    pool_xq = _pool(tc, name="xq", bufs=1)
    xq = pool_xq.tile([P, DB, TOK], f32r, tag="xq")
    for db in range(DB):
        nc.sync.dma_start(xq[:, db, :], r3(xTq_d)[:, db, :])
    wq = pool_w.tile([P, DB, D], f32r, tag="w")
    for db in range(DB):
        nc.sync.dma_start(wq[:, db, :], r3(wqT_d)[:, db, :])
    for ft in range(DB):
        for tck in range(NI):
            ps = psum_p.tile([P, 512], f32, tag="pj", name=f"psq{ft}_{tck}")
            for db in range(DB):
                nc.tensor.matmul(
                    ps[:], wq[:, db, ft * P:(ft + 1) * P],
                    xq[:, db, tck * 512:(tck + 1) * 512],
                    start=(db == 0), stop=(db == DB - 1))
            qst = evict.tile([P, 512], f32r, tag="kst", name=f"qst{ft}_{tck}")
            nc.vector.tensor_copy(qst[:], ps[:])
            nc.sync.dma_start(
                qsp_d[ft * P:(ft + 1) * P, tck * 512:(tck + 1) * 512], qst[:])
    _close(pool_xq)
    _close(evict)
    _close(psum_p)
    _close(pool_w)

    # ================= phase 2: attention =================
    kpool = _pool(tc, name="ktile", bufs=4)
    qpool = _pool(tc, name="qtile", bufs=3)
    ppool = _pool(tc, name="ptile", bufs=3)
    rcpool = _pool(tc, name="rc", bufs=2)
    stg = _pool(tc, name="astg", bufs=3)
    psum_s = _pool(tc, name="ps_s", bufs=2, space="PSUM")
    psum_o = _pool(tc, name="ps_o", bufs=4, space="PSUM")

    for it in range(NI):
        isl = slice(it * 512, (it + 1) * 512)
        for hp in range(DB):
            q_t = qpool.tile([P, 512], f32r, tag="qt", name=f"qt{it}_{hp}")
            nc.sync.dma_start(q_t[:], qsp_d[hp * P:(hp + 1) * P, isl])
            po = [psum_o.tile([HD + 1, 512], f32, tag="po", name=f"po{_u}")
                  for _u in range(2)]
            for jp in range(JB // 2):
                kt = [kpool.tile([P, P], f32r, tag="kt", name=f"kt{_u}")
                      for _u in range(2)]
                for u in range(2):
                    jb = 2 * jp + u
                    nc.sync.dma_start(
                        kt[u][:],
                        ksp_d[hp * P:(hp + 1) * P, jb * P:(jb + 1) * P])
                pse = [psum_s.tile([P, 1024], f32, tag="ps_s", name=f"pse{_u}")
                       for _u in range(2)]
                for par in range(2):  # head A/B within the pair
                    hsl = slice(par * HD, (par + 1) * HD)
                    for u in range(2):
                        nc.tensor.matmul(
                            pse[par][:, u * 512:(u + 1) * 512],
                            kt[u][hsl, :], q_t[hsl, :],
                            start=True, stop=True,
                            tile_position=(par * HD, 0))
                pex = [ppool.tile([P, 1024], f32r, tag="pex", name=f"pex{_u}")
                       for _u in range(2)]
                for par in range(2):
                    nc.scalar.activation(pex[par][:], pse[par][:],
                                         AF.Exp, scale=0.125)
                for par in range(2):
                    h = 2 * hp + par
                    for u in range(2):
                        nc.tensor.matmul(
                            po[par][:], v_aug[:, 2 * jp + u, h, :],
                            pex[par][:, u * 512:(u + 1) * 512],
                            start=(jp == 0 and u == 0),
                            stop=(jp == JB // 2 - 1 and u == 1))
            for par in range(2):
                rc = rcpool.tile([1, 512], f32r, tag="rc", name=f"rc{par}")
                nc.vector.reciprocal(rc[:], po[par][HD:HD + 1, :])
                bc = psum_o.tile([HD, 512], f32, tag="po", name=f"bc{par}")
                nc.tensor.matmul(bc[:], onesr[0:1, 0:HD], rc[:],
                                 start=True, stop=True)
                ast = stg.tile([HD, 512], f32, tag="ast", name=f"ast{par}")
                nc.vector.tensor_copy(ast[:], po[par][0:HD, :])
                nc.vector.tensor_mul(ast[:], ast[:], bc[:])
                frow = hp * P + par * HD
                nc.sync.dma_start(asp_d[frow:frow + HD, isl], ast[:])

    _close(psum_o)
    _close(psum_s)
    _close(stg)
    _close(rcpool)
    _close(ppool)
    _close(qpool)
    _close(kpool)
    _close(pool_v)

    # ================= phase 3: residual + LN1 + FFN + LN2 =================
    rows = _pool(tc, name="rows", bufs=1)
    tmp_pool = _pool(tc, name="ffn_tmp", bufs=3)
    psum_f = _pool(tc, name="ps_f", bufs=4, space="PSUM")
    psum_st = _pool(tc, name="ps_st", bufs=2, space="PSUM")
    psum_bc = _pool(tc, name="ps_bc", bufs=2, space="PSUM")

    def ln_feature_major(src, g_col, e_col, write_out, pfx):
        """LayerNorm over the feature (partition-block) axis of
        src [P, DB, TOK] (f32r).  write_out(db, t, g, e) consumes the
        centered+scaled [P, TOK] f32 block."""
        stm = rows.tile([1, TOK], f32r, tag="lnstm", name=f"{pfx}stm")
        str_ = rows.tile([1, TOK], f32r, tag="lnstr", name=f"{pfx}str")
        m2 = rows.tile([1, TOK], f32, tag="lnm2", name=f"{pfx}m2")
        for si in range(2):
            for ch in range(NI):
                csl = slice(ch * 512, (ch + 1) * 512)
                pm = psum_st.tile([1, 512], f32, tag="ps_st",
                                  name=f"{pfx}pm{si}_{ch}")
                for db in range(DB):
                    if si == 0:
                        rhs = src[:, db, csl]
                    else:
                        sq = tmp_pool.tile([P, 512], f32r, tag="lnsq",
                                           name=f"{pfx}sq{ch}_{db}")
                        nc.vector.tensor_mul(sq[:], src[:, db, csl],
                                             src[:, db, csl])
                        rhs = sq[:]
                    nc.tensor.matmul(pm[:], onesc[:], rhs,
                                     start=(db == 0), stop=(db == DB - 1))
                if si == 0:
                    nc.vector.tensor_scalar_mul(stm[0:1, csl], pm[:], 1.0 / D)
                else:
                    nc.vector.tensor_scalar_mul(m2[0:1, csl], pm[:], 1.0 / D)
        # var = m2 - mean^2 ; rstd = 1/sqrt(var+eps)
        var = rows.tile([1, TOK], f32, tag="lnvar", name=f"{pfx}var")
        nc.vector.tensor_mul(var[:], stm[0:1, :], stm[0:1, :])
        nc.vector.tensor_sub(var[:], m2[:], var[:])
        nc.scalar.activation(var[:], var[:], AF.Sqrt, bias=eps_t[:])
        nc.vector.reciprocal(str_[0:1, :], var[:])
        bcs = tmp_pool.tile([P, 2, TOK], f32, tag="lnbc", name=f"{pfx}bcs")
        for si in range(2):
            st_row = stm if si == 0 else str_
            for ch in range(NI):
                csl = slice(ch * 512, (ch + 1) * 512)
                pb = psum_bc.tile([P, 512], f32, tag="ps_bc",
                                  name=f"{pfx}pb{si}_{ch}")
                nc.tensor.matmul(pb[:], onesr[:], st_row[0:1, csl],
                                 start=True, stop=True)
                nc.vector.tensor_copy(bcs[:, si, csl], pb[:])
        for db in range(DB):
            t = tmp_pool.tile([P, TOK], f32, tag="lnt", name=f"{pfx}t{db}")
            nc.vector.tensor_sub(t[:], src[:, db, :], bcs[:, 0, :])
            nc.vector.tensor_mul(t[:], t[:], bcs[:, 1, :])
            write_out(db, t, g_col, e_col)

    # s1 = x + attn  (f32r; rounds for the stats matmuls)
    pool_h = _pool(tc, name="h", bufs=1)
    h_sb = pool_h.tile([P, DB, TOK], f32r, tag="h")
    pool_s1 = _pool(tc, name="s1", bufs=1)
    s1 = pool_s1.tile([P, DB, TOK], f32r, tag="s1")
    pool_xr = _pool(tc, name="xr", bufs=1)
    xr = pool_xr.tile([P, DB, TOK], f32, tag="xr")
    for db in range(DB):
        nc.sync.dma_start(xr[:, db, :], r3(xTr_d)[:, db, :])
    for db in range(DB):
        at = tmp_pool.tile([P, TOK], f32, tag="lnt", name=f"at{db}")
        nc.sync.dma_start(at[:], asp_d[db * P:(db + 1) * P, :])
        nc.vector.tensor_add(s1[:, db, :], at[:], xr[:, db, :])
    _close(pool_xr)

    def write_h(db, t, g_col, e_col):
        nc.vector.tensor_scalar(
            out=h_sb[:, db, :], in0=t[:], scalar1=g_col[:, db:db + 1],
            scalar2=e_col[:, db:db + 1], op0=ALU.mult, op1=ALU.add)

    ln_feature_major(s1, g1s, e1s, write_h, "ln1")
    _close(pool_s1)

    # ff1 = relu(l1 @ h + b1), feature-major [f1, t]
    pool_s2 = _pool(tc, name="s2", bufs=1)
    s2 = pool_s2.tile([P, DB, TOK], f32r, tag="s2")
    pool_w2 = _pool(tc, name="wts2", bufs=1)
    pool_ff1 = _pool(tc, name="ff1", bufs=1)
    ff1 = pool_ff1.tile([P, DB, TOK], f32r, tag="ff1")
    l1w = pool_w2.tile([P, DB, D], f32r, tag="w2")
    for db in range(DB):
        nc.sync.dma_start(l1w[:, db, :], r3(l1T_d)[:, db, :])
    for f1 in range(DB):
        for tck in range(NI):
            csl = slice(tck * 512, (tck + 1) * 512)
            ps = psum_f.tile([P, 512], f32, tag="ps_f", name=f"pf1_{f1}_{tck}")
            for db in range(DB):
                nc.tensor.matmul(ps[:], l1w[:, db, f1 * P:(f1 + 1) * P],
                                 h_sb[:, db, csl],
                                 start=(db == 0), stop=(db == DB - 1))
            nc.vector.tensor_scalar(
                out=ff1[:, f1, csl], in0=ps[:], scalar1=b1s[:, f1:f1 + 1],
                scalar2=0.0, op0=ALU.add, op1=ALU.max)

    # s2 = h + ff1 @ l2 + b2, feature-major
    l2w = pool_w2.tile([P, DB, D], f32r, tag="w2")
    for db in range(DB):
        nc.sync.dma_start(l2w[:, db, :], r3(l2T_d)[:, db, :])
    for f2 in range(DB):
        for tck in range(NI):
            csl = slice(tck * 512, (tck + 1) * 512)
            ps = psum_f.tile([P, 512], f32, tag="ps_f", name=f"pf2_{f2}_{tck}")
            for f1 in range(DB):
                nc.tensor.matmul(ps[:], l2w[:, f1, f2 * P:(f2 + 1) * P],
                                 ff1[:, f1, csl],
                                 start=(f1 == 0), stop=(f1 == DB - 1))
            ssl = s2[:, f2, csl]
            nc.vector.tensor_scalar_add(ssl, ps[:], b2s[:, f2:f2 + 1])
            nc.vector.tensor_add(ssl, ssl, h_sb[:, f2, csl])

    _close(pool_ff1)
    _close(pool_w2)

    out_pool = _pool(tc, name="outst", bufs=3)

    def write_out(db, t, g_col, e_col):
        o = out_pool.tile([P, TOK], f32, tag="ost", name=f"ost{db}")
        nc.vector.tensor_scalar(
            out=o[:], in0=t[:], scalar1=g_col[:, db:db + 1],
            scalar2=e_col[:, db:db + 1], op0=ALU.mult, op1=ALU.add)
        nc.sync.dma_start(out_d[db * P:(db + 1) * P, :], o[:])

    ln_feature_major(s2, g2s, e2s, write_out, "ln2")

    _close(out_pool)
    _close(pool_s2)
    _close(pool_h)
    _close(psum_bc)
    _close(psum_st)
    _close(psum_f)
    _close(tmp_pool)
    _close(rows)
    _close(const)


def _striped(v):
    return np.ascontiguousarray(np.asarray(v).reshape(DB, P).T,
                                dtype=np.float32)


def kernel(x, wq, wk, wv, ln1_g, ln1_b, l1_w, l1_b, l2_w, l2_b, ln2_g, ln2_b):
    from concourse.bass_utils import run_bass_kernel_spmd

    if "nc" not in _cache:
        _cache["nc"] = _build()
    nc = _cache["nc"]

    x = np.asarray(x, dtype=np.float32)
    c = np.ascontiguousarray
    shared = {
        "wqT": c(np.asarray(wq, np.float32).T),
        "wkT": c(np.asarray(wk, np.float32).T),
        "wvT": c(np.asarray(wv, np.float32).T),
        "l1T": c(np.asarray(l1_w, np.float32).T),
        "l2T": c(np.asarray(l2_w, np.float32).T),
        "onesr": np.ones((1, P), np.float32),
        "onesc": np.ones((P, 1), np.float32),
        "b1s": _striped(l1_b),
        "b2s": _striped(l2_b),
        "g1s": _striped(ln1_g),
        "e1s": _striped(ln1_b),
        "g2s": _striped(ln2_g),
        "e2s": _striped(ln2_b),
    }
    in_maps = []
    for core in range(N_CORES):
        b, p = core // 2, core % 2
        xT = c(x[b].T)
        xTo = c(xT[:, p * TOK:(p + 1) * TOK])
        m = dict(shared)
        m["xT"] = xT
        m["xTq"] = xTo
        m["xTr"] = xTo
        in_maps.append(m)

    res = run_bass_kernel_spmd(nc, in_maps, core_ids=list(range(N_CORES)))
    _cache["last_result"] = res

    y = np.empty((B, S, D), dtype=np.float32)
    for core in range(N_CORES):
        b, p = core // 2, core % 2
        y[b, p * TOK:(p + 1) * TOK, :] = res.results[core]["outT"].T
    return y
